# revision 1
# baseline (speedup 1.0000x reference)
"""CRF loss (sum of gold-path score minus log-partition) Bass/Tile kernel for TRN2.

Problem: B=512, S=512, T=128 CRF loss_fn; out = sum_b [score_b - logZ_b].

Sharding: data-parallel over batch, 64 batches per NeuronCore; host only
slices inputs, reshapes 1-D params to (T,1), and sums 8 per-core scalars.

Denominator per core: exp-domain forward recurrence in tag-major layout
p[(tag)=128 partitions, (batch)=64 free]:
    p_0 = exp(em_0 + start)                       (ACT exp, per-partition bias)
    p_s = (p_{s-1} @ exp(trans)) * exp(em_s - C*) (PE matmul + DVE mult)
C* = E[logsumexp(em)] = 5.3455 folded into the bulk exp as a constant bias;
true sum-renormalization every 64 steps (ones-matmul broadcast + reciprocal)
accumulates log-scales. exp(trans) in [0.9,1.1] keeps everything in fp32 range.

Numerator (mask is all-ones per the spec): gold-path score computed with
PSUM-accumulated matmul statistics instead of element gathers (HW indirect
DMA only supports one offset per partition):
  - one-hot rows OH[(b,s)] = eye128[tags[b,s]] gathered from a constant eye
    table in DRAM (row-gather, one offset/partition, 128 (b,s) pairs/instr)
  - emission term  = trace( sum_pairs OH^T @ em_rows )  (PSUM accumulate)
  - transition term = < sum_pairs OHprev^T @ OHnext , transitions >  (bigram
    counts), plus 15 chunk-boundary pairs via direct element gathers
  - start/end terms via single-offset gathers.
"""

import numpy as np

B, S, T = 512, 512, 128
NCORES = 8
BL = B // NCORES  # 64 batches per core

CSTAR = 5.3455          # E[log sum_j exp(em_j)] for T=128 iid N(0,1) emissions
RENORM_EVERY = 64       # true renormalization cadence (steps)
S_CHUNK = 64            # emission steps per DMA chunk (2 half-DMAs of 32)
HC = S_CHUNK // 2       # steps per half-chunk (partition group)
GROUP = 8               # steps per transpose/exp group (one PSUM bank)

DEBUG = False
VARIANT = 'full'  # full | num_only | den_only | den_copy

_CACHE = {}


def _build_nc(reps=1):
    import concourse.bass as bass
    import concourse.bacc as bacc
    import concourse.tile as tile
    from concourse import mybir
    from concourse.masks import make_identity

    f32 = mybir.dt.float32
    i32 = mybir.dt.int32
    AF = mybir.ActivationFunctionType
    AX = mybir.AxisListType

    nc = bacc.Bacc(
        "TRN2",
        target_bir_lowering=False,
        debug=False,
        enable_asserts=False,
        num_devices=NCORES,
    )

    em_d = nc.dram_tensor("emissions", (BL, S, T), f32, kind="ExternalInput")
    tags_d = nc.dram_tensor("tags", (BL, S), i32, kind="ExternalInput")
    mask_d = nc.dram_tensor("mask", (BL, S), i32, kind="ExternalInput")
    start_d = nc.dram_tensor("start_transitions", (T, 1), f32, kind="ExternalInput")
    end_d = nc.dram_tensor("end_transitions", (T, 1), f32, kind="ExternalInput")
    trans_d = nc.dram_tensor("transitions", (T, T), f32, kind="ExternalInput")
    eye_d = nc.dram_tensor("eyetab", (T, T), f32, kind="ExternalInput")
    out_d = nc.dram_tensor("partial", (1, 1), f32, kind="ExternalOutput")
    dbg = {}
    if DEBUG:
        for nm, shp in [("dbg_emtot", (1, 1)), ("dbg_trtot", (1, 1)),
                        ("dbg_btot", (64, 1)), ("dbg_cacc", (1, 64)),
                        ("dbg_logsw", (1, 64)), ("dbg_pfinal", (128, 64)),
                        ("dbg_emacc", (128, 128)), ("dbg_tracc", (128, 128))]:
            dbg[nm] = nc.dram_tensor(nm, shp, f32, kind="ExternalOutput")

    from contextlib import ExitStack

    n_chunks = S // S_CHUNK
    n_pairs = S // 2          # (c, j) pair indices; 2 steps per pair

    with tile.TileContext(nc) as tc, ExitStack() as ctx:
        consts = ctx.enter_context(tc.tile_pool(name="consts", bufs=1))
        em_pool = ctx.enter_context(tc.tile_pool(name="em", bufs=2))
        e_pool = ctx.enter_context(tc.tile_pool(name="E", bufs=3))
        p_pool = ctx.enter_context(tc.tile_pool(name="p", bufs=3))
        oh_pool = ctx.enter_context(tc.tile_pool(name="oh", bufs=4))
        small = ctx.enter_context(tc.tile_pool(name="small", bufs=2))
        num_pool = ctx.enter_context(tc.tile_pool(name="num", bufs=1))
        r_psum = ctx.enter_context(tc.tile_pool(name="rps", bufs=2, space="PSUM"))
        t_psum = ctx.enter_context(tc.tile_pool(name="tps", bufs=2, space="PSUM"))
        m_psum = ctx.enter_context(tc.tile_pool(name="mps", bufs=2, space="PSUM"))
        g_psum = ctx.enter_context(tc.tile_pool(name="gps", bufs=1, space="PSUM"))

        # ---------------- constants ----------------
        # identity for the PE transposes, valid at both partition halves
        eye2 = consts.tile([128, 64], f32, tag="eye2")
        make_identity(nc, eye2[0:64, :])
        nc.sync.dma_start(eye2[64:128, :], eye2[0:64, :])

        eyesb = consts.tile([128, 128], f32, tag="eyesb")
        nc.sync.dma_start(eyesb[:], eye_d[:])

        ones = consts.tile([128, 128], f32, tag="ones")
        nc.vector.memset(ones[:], 1.0)

        trans_sb = consts.tile([128, 128], f32, tag="trans")
        nc.sync.dma_start(trans_sb[:], trans_d[:])
        mexp = consts.tile([128, 128], f32, tag="mexp")
        nc.scalar.activation(mexp[:], trans_sb[:], AF.Exp)

        startv = consts.tile([128, 1], f32, tag="startv")
        nc.sync.dma_start(startv[:], start_d[:])
        endv = consts.tile([128, 1], f32, tag="endv")
        nc.sync.dma_start(endv[:], end_d[:])
        eexp = consts.tile([128, 1], f32, tag="eexp")
        nc.scalar.activation(eexp[:], endv[:], AF.Exp)

        cacc = consts.tile([1, 64], f32, tag="cacc")
        negc = consts.tile([128, 1], f32, tag="negc")
        nc.vector.memset(negc[:], -CSTAR)

        for _rep in range(reps):
            nc.vector.memset(cacc[:], 0.0)
            # ---------------- numerator setup ----------------
            tags_sb = num_pool.tile([BL, S], i32, tag="tags")
            nc.sync.dma_start(tags_sb[:], tags_d[:])

            # tags2[b + 64h, c*HC + j] = tags[b, c*S_CHUNK + HC*h + j]
            tags2 = num_pool.tile([128, n_pairs], i32, tag="tags2")
            tags_v = tags_d[:].rearrange("b (c t) -> b c t", t=S_CHUNK)
            t2_v = tags2[:].rearrange("p (c j) -> p c j", j=HC)
            nc.sync.dma_start(t2_v[0:64, :, :], tags_v[:, :, 0:HC])
            nc.sync.dma_start(t2_v[64:128, :, :], tags_v[:, :, HC:S_CHUNK])

            # boundary transition pairs: s = 31 + 32k -> s+1, k = 0..14
            tk = tags_sb[:].rearrange("b (k x) -> b k x", x=HC)
            bnd_a = num_pool.tile([BL, 15], i32, tag="bnda")
            nc.gpsimd.tensor_scalar_mul(bnd_a[:], tk[:, 0:15, HC - 1], T)
            bnd_off = num_pool.tile([BL, 15], i32, tag="bndoff")
            nc.gpsimd.tensor_add(bnd_off[:], bnd_a[:], tk[:, 1:16, 0])

            trbnd = num_pool.tile([BL, 15], f32, tag="trbnd")
            for k in range(15):
                nc.gpsimd.indirect_dma_start(
                    out=trbnd[:, k : k + 1],
                    out_offset=None,
                    in_=trans_d[:],
                    in_offset=bass.IndirectOffsetOnAxis(
                        ap=bnd_off[:, k : k + 1], axis=1
                    ),
                )
            stg = num_pool.tile([BL, 1], f32, tag="stg")
            nc.gpsimd.indirect_dma_start(
                out=stg[:], out_offset=None, in_=start_d[:],
                in_offset=bass.IndirectOffsetOnAxis(ap=tags_sb[:, 0:1], axis=0),
            )
            eng = num_pool.tile([BL, 1], f32, tag="eng")
            nc.gpsimd.indirect_dma_start(
                out=eng[:], out_offset=None, in_=end_d[:],
                in_offset=bass.IndirectOffsetOnAxis(ap=tags_sb[:, S - 1 : S], axis=0),
            )

            trbsum = num_pool.tile([BL, 1], f32, tag="trbsum")
            nc.vector.reduce_sum(trbsum[:], trbnd[:], axis=AX.X)
            bs0 = num_pool.tile([BL, 1], f32, tag="bs0")
            nc.vector.tensor_add(bs0[:], stg[:], eng[:])
            bsum = num_pool.tile([BL, 1], f32, tag="bsum")
            nc.vector.tensor_add(bsum[:], bs0[:], trbsum[:])

            emacc = g_psum.tile([128, 128], f32, tag="emacc")
            tracc = g_psum.tile([128, 128], f32, tag="tracc")

            # ---------------- main loop ----------------
            p_cur = None
            oh_tiles = {}
            for c in range(n_chunks):
                em2 = em_pool.tile([128, HC * T], f32, tag="em")
                nc.sync.dma_start(
                    em2[0:64, :],
                    em_d[:, c * S_CHUNK : c * S_CHUNK + HC, :].rearrange(
                        "b s t -> b (s t)"
                    ),
                )
                nc.sync.dma_start(
                    em2[64:128, :],
                    em_d[:, c * S_CHUNK + HC : (c + 1) * S_CHUNK, :].rearrange(
                        "b s t -> b (s t)"
                    ),
                )

                # one-hot gathers + gather-matmuls for this chunk's pairs
                for j in range(HC):
                    pair = c * HC + j
                    oh = oh_pool.tile([128, 128], f32, tag="oh")
                    nc.gpsimd.indirect_dma_start(
                        out=oh[:], out_offset=None, in_=eye_d[:],
                        in_offset=bass.IndirectOffsetOnAxis(
                            ap=tags2[:, pair : pair + 1], axis=0
                        ),
                    )
                    oh_tiles[pair] = oh
                    nc.tensor.matmul(
                        emacc[:], oh[:], em2[:, j * T : (j + 1) * T],
                        start=(pair == 0), stop=(pair == n_pairs - 1),
                        skip_group_check=True,
                    )
                    if j > 0:
                        nc.tensor.matmul(
                            tracc[:], oh_tiles[pair - 1][:], oh[:],
                            start=(pair == 1), stop=(pair == n_pairs - 1),
                            skip_group_check=True,
                        )
                        del oh_tiles[pair - 1]

                # denominator recurrence over this chunk
                for g in range(S_CHUNK // GROUP):
                    emt = t_psum.tile([128, GROUP * 64], f32, tag="emt")
                    for k in range(GROUP):
                        l = g * GROUP + k
                        h, j = l // HC, l % HC
                        nc.tensor.transpose(
                            emt[:, k * 64 : (k + 1) * 64],
                            em2[h * 64 : (h + 1) * 64, j * T : (j + 1) * T],
                            eye2[h * 64 : (h + 1) * 64, :],
                        )
                    e_tile = e_pool.tile([128, GROUP, 64], f32, tag="E")
                    if c == 0 and g == 0:
                        p0 = p_pool.tile([128, 64], f32, tag="p")
                        nc.scalar.activation(
                            p0[:], emt[:, 0:64], AF.Exp, bias=startv[:], scale=1.0
                        )
                        nc.scalar.activation(
                            e_tile[:, 1:GROUP, :], emt[:, 64 : GROUP * 64],
                            AF.Exp, bias=negc[:], scale=1.0,
                        )
                        p_cur = p0
                    else:
                        nc.scalar.activation(
                            e_tile[:], emt[:], AF.Exp, bias=negc[:], scale=1.0
                        )
                    for k in range(GROUP):
                        s = c * S_CHUNK + g * GROUP + k
                        if s == 0:
                            continue
                        r = r_psum.tile([128, 64], f32, tag="r")
                        nc.tensor.matmul(r[:], mexp[:], p_cur[:], start=True, stop=True)
                        p_nxt = p_pool.tile([128, 64], f32, tag="p")
                        if VARIANT == 'den_copy':
                            nc.vector.tensor_copy(p_nxt[:], r[:])
                        else:
                            nc.vector.tensor_mul(p_nxt[:], r[:], e_tile[:, k, :])
                        p_cur = p_nxt
                        if s % RENORM_EVERY == 0:
                            sums = m_psum.tile([128, 64], f32, tag="misc")
                            nc.tensor.matmul(
                                sums[:], ones[:], p_cur[:], start=True, stop=True
                            )
                            inv_s = small.tile([128, 64], f32, tag="invs")
                            nc.vector.reciprocal(inv_s[:], sums[:])
                            p_rn = p_pool.tile([128, 64], f32, tag="p")
                            nc.vector.tensor_mul(p_rn[:], p_cur[:], inv_s[:])
                            log_s = small.tile([1, 64], f32, tag="logs")
                            nc.scalar.activation(log_s[:], sums[0:1, :], AF.Ln)
                            nc.vector.tensor_add(cacc[:], cacc[:], log_s[:])
                            p_cur = p_rn

            if p_cur is None:
                p_cur = p_pool.tile([128, 64], f32, tag="p")
                nc.vector.memset(p_cur[:], 1.0)
            # ---------------- final assembly ----------------
            # denominator: denom_b = cacc + log(sum_j p_j exp(end_j)) + 511*CSTAR
            w = small.tile([128, 64], f32, tag="w")
            nc.vector.tensor_scalar_mul(w[:], p_cur[:], eexp[:])
            sw = m_psum.tile([128, 64], f32, tag="misc")
            nc.tensor.matmul(sw[:], ones[:], w[:], start=True, stop=True)
            logsw = small.tile([1, 64], f32, tag="logsw")
            nc.scalar.activation(logsw[:], sw[0:1, :], AF.Ln)
            den64 = small.tile([1, 64], f32, tag="den64")
            nc.vector.tensor_add(den64[:], cacc[:], logsw[:])
            densum = small.tile([1, 1], f32, tag="densum")
            nc.vector.reduce_sum(densum[:], den64[:], axis=AX.X)

            # numerator totals
            emdiag = small.tile([128, 128], f32, tag="emdiag")
            if VARIANT in ('full', 'num_only'):
                nc.vector.tensor_mul(emdiag[:], emacc[:], eyesb[:])
            else:
                nc.vector.memset(emdiag[:], 0.0)
            emrow = small.tile([128, 1], f32, tag="emrow")
            nc.vector.reduce_sum(emrow[:], emdiag[:], axis=AX.X)

            trmul = small.tile([128, 128], f32, tag="trmul")
            if VARIANT in ('full', 'num_only'):
                nc.vector.tensor_mul(trmul[:], tracc[:], trans_sb[:])
            else:
                nc.vector.memset(trmul[:], 0.0)
            trrow = small.tile([128, 1], f32, tag="trrow")
            nc.vector.reduce_sum(trrow[:], trmul[:], axis=AX.X)

            sc_ps = m_psum.tile([1, 1], f32, tag="misc")
            nc.tensor.matmul(sc_ps[:], ones[0:128, 0:1], emrow[:],
                             start=True, stop=False, skip_group_check=True)
            nc.tensor.matmul(sc_ps[:], ones[0:128, 0:1], trrow[:],
                             start=False, stop=False, skip_group_check=True)
            nc.tensor.matmul(sc_ps[:], ones[0:64, 0:1], bsum[:],
                             start=False, stop=True, skip_group_check=True)
            score_sb = small.tile([1, 1], f32, tag="score_sb")
            nc.vector.tensor_copy(score_sb[:], sc_ps[:])

            res0 = small.tile([1, 1], f32, tag="res0")
            nc.vector.tensor_sub(res0[:], score_sb[:], densum[:])
            res1 = small.tile([1, 1], f32, tag="res1")
            nc.vector.tensor_scalar_add(res1[:], res0[:], -float((S - 1) * CSTAR * BL))
            nc.sync.dma_start(out_d[:], res1[:])

            if DEBUG:
                nc.sync.dma_start(dbg["dbg_btot"][:], bsum[:])
                nc.sync.dma_start(dbg["dbg_cacc"][:], cacc[:])
                nc.sync.dma_start(dbg["dbg_logsw"][:], logsw[:])
                nc.sync.dma_start(dbg["dbg_pfinal"][:], p_cur[:])
                emacc_cp = small.tile([128, 128], f32, tag="emacc_cp")
                nc.vector.tensor_copy(emacc_cp[:], emacc[:])
                nc.sync.dma_start(dbg["dbg_emacc"][:], emacc_cp[:])
                tracc_cp = small.tile([128, 128], f32, tag="tracc_cp")
                nc.vector.tensor_copy(tracc_cp[:], tracc[:])
                nc.sync.dma_start(dbg["dbg_tracc"][:], tracc_cp[:])
                emt_ps = m_psum.tile([1, 1], f32, tag="misc")
                nc.tensor.matmul(emt_ps[:], ones[0:128, 0:1], emrow[:],
                                 start=True, stop=True, skip_group_check=True)
                emt_sb = small.tile([1, 1], f32, tag="emt_sb")
                nc.vector.tensor_copy(emt_sb[:], emt_ps[:])
                nc.sync.dma_start(dbg["dbg_emtot"][:], emt_sb[:])
                trt_ps = m_psum.tile([1, 1], f32, tag="misc")
                nc.tensor.matmul(trt_ps[:], ones[0:128, 0:1], trrow[:],
                                 start=True, stop=True, skip_group_check=True)
                trt_sb = small.tile([1, 1], f32, tag="trt_sb")
                nc.vector.tensor_copy(trt_sb[:], trt_ps[:])
                nc.sync.dma_start(dbg["dbg_trtot"][:], trt_sb[:])

    nc.compile()
    return nc


def _get_nc(reps=1):
    key = ("nc", reps, VARIANT)
    if key not in _CACHE:
        _CACHE[key] = _build_nc(reps)
    return _CACHE[key]


_EYE = None


def _make_in_maps(emissions, tags, mask, start_transitions, end_transitions,
                  transitions):
    global _EYE
    if _EYE is None:
        _EYE = np.eye(T, dtype=np.float32)
    emissions = np.ascontiguousarray(emissions, dtype=np.float32)
    tags = np.ascontiguousarray(tags, dtype=np.int32)
    mask = np.ascontiguousarray(mask, dtype=np.int32)
    start = np.ascontiguousarray(start_transitions, dtype=np.float32).reshape(T, 1)
    end = np.ascontiguousarray(end_transitions, dtype=np.float32).reshape(T, 1)
    trans = np.ascontiguousarray(transitions, dtype=np.float32)

    in_maps = []
    for core in range(NCORES):
        sl = slice(core * BL, (core + 1) * BL)
        in_maps.append(
            {
                "emissions": np.ascontiguousarray(emissions[sl]),
                "tags": np.ascontiguousarray(tags[sl]),
                "mask": np.ascontiguousarray(mask[sl]),
                "start_transitions": start,
                "end_transitions": end,
                "transitions": trans,
                "eyetab": _EYE,
            }
        )
    return in_maps


def kernel_run(inputs, trace=False, reps=1, **kw):
    from concourse.bass_utils import run_bass_kernel_spmd

    nc = _get_nc(reps)
    in_maps = _make_in_maps(**inputs)
    res = run_bass_kernel_spmd(
        nc, in_maps, core_ids=list(range(NCORES)), trace=trace, **kw
    )
    partials = [r["partial"].reshape(()) for r in res.results]
    total = np.float32(np.sum(np.asarray(partials, dtype=np.float64)))
    return total, res


def kernel(**inputs):
    total, _ = kernel_run(inputs, trace=False)
    return total

